# revision 13
# baseline (speedup 1.0000x reference)
"""DriftAwareLightMemory fused Bass/Tile kernel for 8 trn2 NeuronCores.

Strategy ((batch, L-half) sharded, feature-major, fp8 DoubleRow memory path):
  - Core k owns batch b = k//2 and sequence half h = k%2 (512 of the 1024
    L rows).  All device tensors are feature-major (FM: [d-partition, l],
    D split in 4 chunks of 128), pre-transposed on the host.
  - memory_snapshot ships as ONE contiguous fp8-e4m3 tile [128, 16*2048]
    (t-major per partition) so the input DMA runs with 8KB/partition
    descriptors; slot 15 additionally ships bf16 (feeds delta / x_phys).
  - enhanced = sum_t attn_t * mem[t] runs on the tensor engine as fp8
    *DoubleRow* diag-matmuls: slots are paired (2q, 2q+1) along the
    contraction dim, halving PE rows (2816 -> 2048 cycles per chunk per
    pass).  A warmup pass accumulates sum_t mem[t]/16 into held PSUM banks
    during the collective window; post-softmax (attn-1/16) corrections
    continue the same accumulation group.  Stationaries are scaled x64
    (fp8 denormal floor), 1/64 folded into the drain.
  - Column sums over L (t-means) are split across vector reduces, scalar
    Copy-activations with accum_out, and gpsimd reduces, writing straight
    into the bf16 AllReduce payload.  q_global / cur_drift use linearity:
    sum_l raw = Wo^T sum_l mid + L*b, with Wq/L, (Wo Wq)/L and Wcd/L folded
    host-side so the payload projections are single accumulation chains.
  - Two pipelined 2-core-pair AllReduces (DRAM bounce): AR-A carries the 64
    colsum columns (dispatches as soon as the mem DMAs + colsums land),
    AR-B the 8 projected q/cd columns (right after phase A's mid).
  - pos_emb is input-independent -> fully host-precomputed (seq_W folded).
  - softmax(16) uses a cubic exp approximation (scores are ~1e-1).
  - fuse: f1 logits (pre-AR) + f2@enh PSUM, sigmoid on scalar, final
    x + raw + g*enh assembled on vector/gpsimd, output written bf16 FM.

kernel(**inputs) takes full-size numpy inputs, returns [4,1024,512] float32.
"""
import sys
import math

sys.path.insert(0, "/opt/trn_rl_repo")

import numpy as np
import ml_dtypes

import concourse.bass as bass
import concourse.bacc as bacc
import concourse.tile as tile
from concourse import bass_utils, mybir

dt = mybir.dt
AF = mybir.ActivationFunctionType
ALU = mybir.AluOpType
AX = mybir.AxisListType
PM = mybir.MatmulPerfMode

B, T, L, D = 4, 16, 1024, 512
NC = 8
LH = L // 2             # 512 L rows per core
NCH = 4                 # feature chunks of 128
LAMBDA = 0.3
C_CONT = 1.0 / math.sqrt(D)
C_DRIFT = -LAMBDA / D
INV_L = 1.0 / L

WN = ["wx", "wpn", "wd", "gx", "gp", "wo", "wq", "wqo", "wcd",
      "wm", "wmd", "f1", "f2"]
WOFF = {n: i * 2048 for i, n in enumerate(WN)}

BN = ["b_t1", "b_Ap", "gate_b", "outp_b", "qcb_half", "cdb_half",
      "mem_b", "memd_b", "fuse_b"]
BI = {n: i for i, n in enumerate(BN)}
PE_COL0 = 4 * len(BN)               # pe_fm starts after biases in BIASPE

_CACHE = {}


def _wdev(w):
    """[512,512] weight -> [128,2048] device layout (k-chunk c at cols c*512)."""
    return np.ascontiguousarray(
        w.reshape(4, 128, 512).transpose(1, 0, 2).reshape(128, 2048))


def _fm(v):
    """[512] vector -> [128,4] feature-major bias columns."""
    return np.ascontiguousarray(v.reshape(4, 128).T)


def _pe_table(seq_W, seq_b):
    pos = np.arange(1, T + 1, dtype=np.float32)
    half = D // 2
    div = np.exp(-math.log(10000.0) * (2.0 * np.arange(half, dtype=np.float32) / D))
    ang = pos[:, None] * div
    pe = np.stack([np.sin(ang), np.cos(ang)], axis=-1).reshape(T, D)
    pe = pe.astype(np.float32) @ seq_W + seq_b          # [T, D]
    # -> [128, 64], col c*16 + t
    return np.ascontiguousarray(
        pe.T.reshape(4, 128, T).transpose(1, 0, 2).reshape(128, 4 * T))


def _bf(x):
    return np.asarray(x, np.float32).astype(ml_dtypes.bfloat16)


def _build():
    nc = bacc.Bacc("TRN2", target_bir_lowering=False, debug=False,
                   num_devices=NC)
    f32, bf16 = dt.float32, dt.bfloat16
    f8 = dt.float8e4

    MEMF8 = nc.dram_tensor("MEMF8", [128, T * 2048], f8, kind="ExternalInput").ap()
    M15D = nc.dram_tensor("M15D", [128, 2048], bf16, kind="ExternalInput").ap()
    XT = nc.dram_tensor("XT", [128, 2048], bf16, kind="ExternalInput").ap()
    WALL = nc.dram_tensor("WALL", [128, len(WN) * 2048], bf16,
                          kind="ExternalInput").ap()
    BIASPE = nc.dram_tensor("BIASPE", [128, PE_COL0 + 64], f32,
                            kind="ExternalInput").ap()
    CONSTB = nc.dram_tensor("CONSTB", [128, 2], bf16, kind="ExternalInput").ap()
    IDENT8 = nc.dram_tensor("IDENT8", [128, 128], f8, kind="ExternalInput").ap()
    ONESB = nc.dram_tensor("ONESB", [1, 128], bf16, kind="ExternalInput").ap()
    OUT = nc.dram_tensor("OUT", [NCH, 128, LH], bf16, kind="ExternalOutput").ap()

    groups = [[2 * b, 2 * b + 1] for b in range(B)]

    def _emit(tc):
        with tc.tile_pool(name="sb", bufs=1) as sb, \
             tc.tile_pool(name="ps", bufs=1, space="PSUM") as ps, \
             tc.tile_pool(name="dram", bufs=1, space="DRAM") as dram:

            def S(shape, dtype, tag, bufs=1):
                return sb.tile(shape, dtype, tag=tag, bufs=bufs, name=tag)

            def PA(cols=512):
                return ps.tile([128, 512], f32, tag="pa", bufs=4,
                               name="pa")[:, 0:cols]

            def TS(eng, out, in0, s1, s2=None, op0=ALU.add, op1=None):
                kw = dict(out=out, in0=in0, scalar1=s1, scalar2=s2, op0=op0)
                if op1 is not None:
                    kw["op1"] = op1
                eng.tensor_scalar(**kw)

            # ---------------- input DMAs ----------------
            constb = S([128, 2], bf16, "constb")
            ident8 = S([128, 128], f8, "ident8")
            onesb = S([1, 128], bf16, "onesb")
            biaspe = S([128, PE_COL0 + 64], f32, "biaspe")
            nc.sync.dma_start(constb, CONSTB)
            nc.sync.dma_start(ident8, IDENT8)
            nc.sync.dma_start(onesb, ONESB)
            nc.sync.dma_start(biaspe, BIASPE)
            xt = S([128, 2048], bf16, "xt")
            nc.sync.dma_start(xt, XT)
            m15 = S([128, 2048], bf16, "m15")
            nc.sync.dma_start(m15, M15D)
            wall = S([128, len(WN) * 2048], bf16, "wall")
            # weight groups released progressively (phase-A first)
            WG = [("wx", "wpn"), ("wd", "gx", "gp"), ("wo", "wq", "wqo", "wcd"),
                  ("wm", "wmd", "f1", "f2")]
            for grp in WG:
                lo = WOFF[grp[0]]
                hi = WOFF[grp[-1]] + 2048
                nc.sync.dma_start(wall[:, lo:hi], WALL[:, lo:hi])

            # memory snapshot on the scalar HWDGE queue (parallel stream)
            memsb = S([128, T * 2048], f8, "memsb")
            for gqi in range(4):
                lo = gqi * 4 * 2048
                nc.scalar.dma_start(memsb[:, lo:lo + 4 * 2048],
                                    MEMF8[:, lo:lo + 4 * 2048])

            ccont = constb[:, 0:1]
            cdrift = constb[:, 1:2]
            memv = memsb.rearrange("p (t c l) -> p t c l", t=T, c=NCH, l=512)

            def bias_col(name, c):
                return biaspe[:, 4 * BI[name] + c: 4 * BI[name] + c + 1]

            def pe_col(c0, n):
                return biaspe[:, PE_COL0 + c0: PE_COL0 + c0 + n]

            def w_chunk(n, c_k, c_out):
                o = WOFF[n] + c_k * 512 + c_out * 128
                return wall[:, o:o + 128]

            def x_fm(c):
                return xt[:, c * 512:(c + 1) * 512]

            def xp_fm(c):
                return m15[:, c * 512:(c + 1) * 512]

            # AR payloads (bf16)
            payA_in = S([128, 64], bf16, "payA_in")
            payA_out = S([128, 64], bf16, "payA_out")
            payB_in = S([128, 8], bf16, "payB_in")
            payB_out = S([128, 8], bf16, "payB_out")

            # warmup stationary: two interleaved 4*I (= 64/16) fp8 identities
            four8 = S([128, 256], f8, "four8")
            TS(nc.vector, four8[:, 0:128], ident8, 4.0, op0=ALU.mult)
            TS(nc.vector, four8[:, 128:256], ident8, 4.0, op0=ALU.mult)

            # ---------------- colsums (payA) ----------------
            csjunk = S([128, 512], bf16, "csjunk", bufs=2)

            def colsum_s(t, c):
                # one chunk on the scalar engine (Copy + accum_out)
                with nc.allow_low_precision("bf16 colsums; attn tolerant"):
                    nc.scalar.activation(
                        csjunk, memv[:, t, c, :], AF.Copy,
                        accum_out=payA_in[:, 4 * t + c: 4 * t + c + 1])

            def colsum_v(t):
                # all 4 chunks of one t in a single vector reduce
                with nc.allow_low_precision("bf16 colsums; attn tolerant"):
                    nc.vector.reduce_sum(out=payA_in[:, 4 * t:4 * t + 4],
                                         in_=memv[:, t, :, :], axis=AX.X)

            # pool has no free-axis reduce; even t -> scalar, odd t -> vector
            # early scalar batch (before its drain work)
            for t in range(0, 8, 2):
                for c in range(NCH):
                    colsum_s(t, c)

            # ---------------- phase A ----------------
            delta = S([128, 2048], bf16, "delta")
            nc.vector.tensor_tensor(out=delta, in0=xt, in1=m15,
                                    op=ALU.subtract)
            xsum = S([128, 4], bf16, "xsum")
            cs15 = S([128, 4], bf16, "cs15")
            qin_d = S([128, 4], bf16, "qin_d")
            with nc.allow_low_precision("bf16 sums; q path tolerant"):
                nc.vector.reduce_sum(
                    out=xsum, in_=xt.rearrange("p (c l) -> p c l", c=4, l=512),
                    axis=AX.X)
                nc.vector.reduce_sum(
                    out=cs15,
                    in_=m15.rearrange("p (c l) -> p c l", c=4, l=512),
                    axis=AX.X)
                nc.vector.tensor_copy(payA_in[:, 60:64], cs15)
                # dsum = sum_l delta = xsum - colsum(m15)
                nc.vector.tensor_tensor(out=qin_d, in0=xsum, in1=cs15,
                                        op=ALU.subtract)

            # t1 = x@Wx + xph@Wpn + b_t1
            t1 = S([128, 2048], bf16, "t1")
            for c in range(NCH):
                psum = PA()
                for ck in range(NCH):
                    nc.tensor.matmul(psum, w_chunk("wx", ck, c), x_fm(ck),
                                     start=(ck == 0), stop=False)
                for ck in range(NCH):
                    nc.tensor.matmul(psum, w_chunk("wpn", ck, c), xp_fm(ck),
                                     start=False, stop=(ck == NCH - 1))
                nc.scalar.activation(t1[:, c * 512:(c + 1) * 512], psum,
                                     AF.Identity, bias=bias_col("b_t1", c))

            # A' = delta@Wd + b_Ap   (into `mid`, finished in place)
            mid = S([128, 2048], bf16, "mid")
            for c in range(NCH):
                psum = PA()
                for ck in range(NCH):
                    nc.tensor.matmul(psum, w_chunk("wd", ck, c),
                                     delta[:, ck * 512:(ck + 1) * 512],
                                     start=(ck == 0), stop=(ck == NCH - 1))
                nc.scalar.activation(mid[:, c * 512:(c + 1) * 512], psum,
                                     AF.Identity, bias=bias_col("b_Ap", c))

            # g = sigmoid(x@Gx + xph@Gp + gate_b)
            g = S([128, 2048], bf16, "g")
            for c in range(NCH):
                psum = PA()
                for ck in range(NCH):
                    nc.tensor.matmul(psum, w_chunk("gx", ck, c), x_fm(ck),
                                     start=(ck == 0), stop=False)
                for ck in range(NCH):
                    nc.tensor.matmul(psum, w_chunk("gp", ck, c), xp_fm(ck),
                                     start=False, stop=(ck == NCH - 1))
                nc.scalar.activation(g[:, c * 512:(c + 1) * 512], psum,
                                     AF.Sigmoid, bias=bias_col("gate_b", c))

            # mid = t1 + g*(A' - t1)   (in place; last chunk on gpsimd)
            for c in range(NCH):
                sl = slice(c * 512, (c + 1) * 512)
                eng = nc.vector
                eng.tensor_tensor(out=mid[:, sl], in0=mid[:, sl],
                                  in1=t1[:, sl], op=ALU.subtract)
                eng.tensor_tensor(out=mid[:, sl], in0=mid[:, sl],
                                  in1=g[:, sl], op=ALU.mult)
                eng.tensor_tensor(out=mid[:, sl], in0=mid[:, sl],
                                  in1=t1[:, sl], op=ALU.add)

            # qin_q = sum_l mid (Wq/L, WoWq/L, Wcd/L folded host-side)
            qin_q = S([128, 4], bf16, "qin_q")
            with nc.allow_low_precision("bf16 sums; q path tolerant"):
                nc.vector.reduce_sum(
                    out=qin_q,
                    in_=mid.rearrange("p (c l) -> p c l", c=4, l=512),
                    axis=AX.X)

            # payB: projected q (cols 0..3) and cd (cols 4..7)
            with nc.allow_low_precision("bf16 AR payload"):
                for c in range(NCH):
                    psq = PA(1)
                    for ck in range(NCH):
                        nc.tensor.matmul(psq, w_chunk("wq", ck, c),
                                         xsum[:, ck:ck + 1],
                                         start=(ck == 0), stop=False)
                    for ck in range(NCH):
                        nc.tensor.matmul(psq, w_chunk("wqo", ck, c),
                                         qin_q[:, ck:ck + 1],
                                         start=False, stop=(ck == NCH - 1))
                    TS(nc.vector, payB_in[:, c:c + 1], psq,
                       bias_col("qcb_half", c))
                for c in range(NCH):
                    psq = PA(1)
                    for ck in range(NCH):
                        nc.tensor.matmul(psq, w_chunk("wcd", ck, c),
                                         qin_d[:, ck:ck + 1],
                                         start=(ck == 0), stop=(ck == NCH - 1))
                    TS(nc.vector, payB_in[:, 4 + c:5 + c], psq,
                       bias_col("cdb_half", c))

            # remaining colsums (vector batched + late scalar batch)
            for t in range(1, 15, 2):
                colsum_v(t)
            for t in range(8, 15, 2):
                for c in range(NCH):
                    colsum_s(t, c)

            # ---------------- AllReduces (DRAM bounce, 2-core pairs) -------
            arA_in = dram.tile([128, 64], bf16, tag="arA_in", name="arA_in")
            arA_out = dram.tile([128, 64], bf16, tag="arA_out", name="arA_out")
            arB_in = dram.tile([128, 8], bf16, tag="arB_in", name="arB_in")
            arB_out = dram.tile([128, 8], bf16, tag="arB_out", name="arB_out")
            nc.sync.dma_start(arA_in, payA_in)
            nc.gpsimd.collective_compute(
                "AllReduce", ALU.add, replica_groups=groups,
                ins=[arA_in[:]], outs=[arA_out[:]])
            nc.sync.dma_start(arB_in, payB_in)
            nc.gpsimd.collective_compute(
                "AllReduce", ALU.add, replica_groups=groups,
                ins=[arB_in[:]], outs=[arB_out[:]])
            nc.sync.dma_start(payA_out, arA_out)
            nc.sync.dma_start(payB_out, arB_out)

            # ---------------- AR-window work ----------------
            # raw = mid@Wo + outp_b
            raw = S([128, 2048], bf16, "raw")
            for c in range(NCH):
                psum = PA()
                for ck in range(NCH):
                    nc.tensor.matmul(psum, w_chunk("wo", ck, c),
                                     mid[:, ck * 512:(ck + 1) * 512],
                                     start=(ck == 0), stop=(ck == NCH - 1))
                nc.scalar.activation(raw[:, c * 512:(c + 1) * 512], psum,
                                     AF.Identity, bias=bias_col("outp_b", c))

            # f1 logits -> SBUF (f2 psum adds later)
            f1log = S([128, 2048], bf16, "f1log")
            for c in range(NCH):
                psum = PA()
                for ck in range(NCH):
                    nc.tensor.matmul(psum, w_chunk("f1", ck, c), x_fm(ck),
                                     start=(ck == 0), stop=(ck == NCH - 1))
                with nc.allow_low_precision("bf16 sigmoid logits"):
                    TS(nc.vector, f1log[:, c * 512:(c + 1) * 512], psum,
                       bias_col("fuse_b", c))

            # warmup: eps[c] = 64 * sum_t mem[t]/16 via DoubleRow pairs
            eps = [ps.tile([128, 512], f32, tag="peps", bufs=4, name="peps")
                   for _ in range(NCH)]
            f8v = four8.rearrange("p (j m) -> p j m", j=2, m=128)
            for q in range(T // 2):
                for c in range(NCH):
                    nc.tensor.matmul(eps[c], f8v, memv[:, 2 * q:2 * q + 2, c, :],
                                     start=(q == 0), stop=False,
                                     perf_mode=PM.DoubleRow)

            # s2 = x + raw
            s2 = S([128, 2048], bf16, "s2")
            nc.vector.tensor_tensor(out=s2, in0=xt, in1=raw, op=ALU.add)

            # ---------------- post-AR: scores ----------------
            po = payA_out.rearrange("p (t c) -> p c t", t=T, c=NCH)
            qgcd = S([128, 8], f32, "qgcd")
            nc.vector.tensor_copy(qgcd, payB_out)
            mean_fm = S([128, 64], bf16, "mean_fm")   # [c*16+t]
            md_fm = S([128, 64], bf16, "md_fm")
            for c in range(NCH):
                nc.vector.scalar_tensor_tensor(
                    out=mean_fm[:, c * 16:c * 16 + 16],
                    in0=po[:, c, :], scalar=INV_L,
                    in1=pe_col(c * 16, 16),
                    op0=ALU.mult, op1=ALU.add)
                nc.vector.tensor_copy(md_fm[:, c * 16:c * 16 + 1],
                                      mean_fm[:, c * 16:c * 16 + 1])
                nc.vector.tensor_tensor(
                    out=md_fm[:, c * 16 + 1:c * 16 + 16],
                    in0=mean_fm[:, c * 16 + 1:c * 16 + 16],
                    in1=mean_fm[:, c * 16:c * 16 + 15], op=ALU.subtract)

            # gm/dm score terms straight from PSUM
            score_ps = ps.tile([128, 512], f32, tag="pa", bufs=4,
                               name="pa")[0:1, 0:16]
            first_sc = [True]

            def score_mm(stat, pr, last):
                nc.tensor.matmul(score_ps, stat, pr,
                                 start=first_sc[0], stop=last)
                first_sc[0] = False

            prs, sqs = [], []
            for c in range(NCH):
                psum = PA(16)
                for ck in range(NCH):
                    nc.tensor.matmul(psum, w_chunk("wm", ck, c),
                                     mean_fm[:, ck * 16:(ck + 1) * 16],
                                     start=(ck == 0), stop=(ck == NCH - 1))
                pr = S([128, 16], bf16, "pr", bufs=2)
                TS(nc.vector, pr, psum, bias_col("mem_b", c),
                   qgcd[:, c:c + 1], op0=ALU.add, op1=ALU.mult)
                prs.append(pr)
            for c in range(NCH):
                psum = PA(16)
                for ck in range(NCH):
                    nc.tensor.matmul(psum, w_chunk("wmd", ck, c),
                                     md_fm[:, ck * 16:(ck + 1) * 16],
                                     start=(ck == 0), stop=(ck == NCH - 1))
                dd = S([128, 16], bf16, "dd", bufs=2)
                TS(nc.vector, dd, psum, bias_col("memd_b", c),
                   qgcd[:, 4 + c:5 + c], op0=ALU.add, op1=ALU.subtract)
                sq = S([128, 16], bf16, "sq", bufs=2)
                nc.vector.tensor_tensor(out=sq, in0=dd, in1=dd, op=ALU.mult)
                sqs.append(sq)
            for c in range(NCH):
                score_mm(ccont, prs[c], False)
            for c in range(NCH):
                score_mm(cdrift, sqs[c], c == NCH - 1)

            # softmax via cubic exp (scores are ~±0.15)
            score = S([1, 16], f32, "score")
            nc.vector.tensor_copy(score, score_ps)
            u = S([1, 16], f32, "sm_u")
            TS(nc.vector, u, score, 1.0 / 6.0, 0.5, op0=ALU.mult, op1=ALU.add)
            v = S([1, 16], f32, "sm_v")
            nc.vector.tensor_tensor(out=v, in0=u, in1=score, op=ALU.mult)
            TS(nc.vector, v, v, 1.0)
            e = S([1, 16], f32, "sm_e")
            nc.vector.tensor_tensor(out=e, in0=v, in1=score, op=ALU.mult)
            TS(nc.vector, e, e, 1.0)
            ssum = S([1, 1], f32, "sm_s")
            nc.vector.reduce_sum(out=ssum, in_=e, axis=AX.X)
            rs = S([1, 1], f32, "sm_r")
            nc.vector.reciprocal(rs, ssum)
            attn_b = S([1, 16], bf16, "attn_b")
            with nc.allow_low_precision("bf16 attn"):
                TS(nc.vector, attn_b, e, rs, op0=ALU.mult)

            # broadcast attn over partitions; abc = (attn - 1/16)*64
            ab_ps = PA(16)
            nc.tensor.matmul(ab_ps, onesb, attn_b, start=True, stop=True)
            abc = S([128, 16], f32, "abc")
            TS(nc.vector, abc, ab_ps, -1.0 / 16.0, 64.0,
               op0=ALU.add, op1=ALU.mult)

            # pc = attn . pe  (per chunk)
            pc_fm = S([128, 4], f32, "pc_fm")
            for c in range(NCH):
                tmp = S([128, 16], f32, "pc_tmp", bufs=2)
                nc.vector.tensor_tensor(out=tmp, in0=pe_col(c * 16, 16),
                                        in1=ab_ps, op=ALU.mult)
                nc.vector.reduce_sum(out=pc_fm[:, c:c + 1], in_=tmp, axis=AX.X)

            # correction stationaries: dgq[q] = diag(abc[2q]) ⊕ diag(abc[2q+1])
            dgqs = []
            for q in range(T // 2):
                dgq = S([128, 256], f8, "dgq", bufs=4)
                with nc.allow_low_precision("fp8 correction stationaries"):
                    nc.scalar.activation(dgq[:, 0:128], ident8, AF.Copy,
                                         scale=abc[:, 2 * q:2 * q + 1])
                    nc.scalar.activation(dgq[:, 128:256], ident8, AF.Copy,
                                         scale=abc[:, 2 * q + 1:2 * q + 2])
                dgqs.append(dgq.rearrange("p (j m) -> p j m", j=2, m=128))

            # ---------------- enhanced + f2 (interleaved on PE) ------------
            enh = S([128, 2048], bf16, "enh")
            fps = [ps.tile([128, 512], f32, tag="pa", bufs=4, name="pa")
                   for _ in range(NCH)]

            def corr_chunk(c):
                for q in range(T // 2):
                    nc.tensor.matmul(eps[c], dgqs[q],
                                     memv[:, 2 * q:2 * q + 2, c, :],
                                     start=False, stop=(q == T // 2 - 1),
                                     perf_mode=PM.DoubleRow)
                # drain: enh = eps/64 + pc  (scalar engine)
                nc.scalar.activation(enh[:, c * 512:(c + 1) * 512], eps[c],
                                     AF.Identity, bias=pc_fm[:, c:c + 1],
                                     scale=1.0 / 64.0)

            def f2_k(ck):
                for c in range(NCH):
                    nc.tensor.matmul(fps[c], w_chunk("f2", ck, c),
                                     enh[:, ck * 512:(ck + 1) * 512],
                                     start=(ck == 0), stop=(ck == NCH - 1))

            corr_chunk(0)
            corr_chunk(1)
            f2_k(0)
            corr_chunk(2)
            f2_k(1)
            corr_chunk(3)
            f2_k(2)
            f2_k(3)

            # ---------------- fuse + output ----------------
            for c in range(NCH):
                sl = slice(c * 512, (c + 1) * 512)
                ful = S([128, 512], bf16, "ful", bufs=2)
                with nc.allow_low_precision("bf16 sigmoid logits"):
                    nc.vector.tensor_tensor(out=ful, in0=fps[c],
                                            in1=f1log[:, sl], op=ALU.add)
                fg = S([128, 512], bf16, "fg", bufs=2)
                nc.scalar.activation(fg, ful, AF.Sigmoid)
                p1 = S([128, 512], bf16, "p1", bufs=2)
                nc.vector.tensor_tensor(out=p1, in0=fg, in1=enh[:, sl],
                                        op=ALU.mult)
                of = S([128, 512], bf16, "of", bufs=2)
                eng = nc.vector
                eng.tensor_tensor(out=of, in0=p1, in1=s2[:, sl], op=ALU.add)
                nc.scalar.dma_start(OUT[c], of)

    with tile.TileContext(nc) as tc:
        _emit(tc)

    nc.compile()
    return nc


def _prep_maps(inputs):
    x = np.asarray(inputs["x"], np.float32)
    mem = np.asarray(inputs["memory_snapshot"], np.float32)

    gw = np.asarray(inputs["gate_W"], np.float32)
    fw = np.asarray(inputs["fuse_W"], np.float32)
    outp_W = np.asarray(inputs["outp_W"], np.float32)
    q_W = np.asarray(inputs["q_W"], np.float32)
    weights = {
        "wx": np.asarray(inputs["xproj_W"], np.float32),
        "wpn": -np.asarray(inputs["phys_W"], np.float32),
        "wd": np.asarray(inputs["delta_W"], np.float32),
        "gx": gw[0:512] + gw[512:1024],
        "gp": gw[1024:1536] - gw[0:512],
        "wo": outp_W,
        "wq": q_W * INV_L,
        "wqo": (outp_W @ q_W) * INV_L,
        "wcd": np.asarray(inputs["curd_W"], np.float32) * INV_L,
        "wm": np.asarray(inputs["mem_W"], np.float32),
        "wmd": np.asarray(inputs["memd_W"], np.float32),
        "f1": fw[0:512],
        "f2": fw[512:1024],
    }
    wall = np.concatenate([_wdev(weights[n]) for n in WN], axis=1)

    b = {k: np.asarray(inputs[k], np.float32) for k in
         ["delta_b", "xproj_b", "phys_b", "gate_b", "outp_b", "q_b",
          "mem_b", "curd_b", "memd_b", "fuse_b", "seq_b"]}
    biaspe = np.zeros((128, PE_COL0 + 64), np.float32)
    bvals = {
        "b_t1": b["xproj_b"] - b["phys_b"],
        "b_Ap": b["delta_b"],
        "gate_b": b["gate_b"],
        "outp_b": b["outp_b"],
        "qcb_half": 0.5 * (b["outp_b"] @ q_W + b["q_b"]),
        "cdb_half": 0.5 * b["curd_b"],
        "mem_b": b["mem_b"],
        "memd_b": b["memd_b"],
        "fuse_b": b["fuse_b"],
    }
    for n, v in bvals.items():
        biaspe[:, 4 * BI[n]:4 * BI[n] + 4] = _fm(v)
    biaspe[:, PE_COL0:] = _pe_table(
        np.asarray(inputs["seq_W"], np.float32), b["seq_b"])

    constb = np.zeros((128, 2), np.float32)
    constb[:, 0] = C_CONT
    constb[:, 1] = C_DRIFT

    shared = {
        "WALL": _bf(wall),
        "BIASPE": np.ascontiguousarray(biaspe),
        "CONSTB": _bf(constb),
        "ONESB": _bf(np.ones((1, 128), np.float32)),
        "IDENT8": np.eye(128, dtype=np.float32).astype(ml_dtypes.float8_e4m3),
    }

    in_maps = []
    for k in range(NC):
        bb, h = k // 2, k % 2
        sl = slice(h * LH, (h + 1) * LH)
        m = dict(shared)
        # FM layouts: [p, c*512 + l] = src[l, c*128 + p]
        xs = x[bb, sl, :]                    # [512 l, 512 d]
        m["XT"] = _bf(np.ascontiguousarray(
            xs.T.reshape(4, 128, LH).transpose(1, 0, 2).reshape(128, 2048)))
        ms = mem[bb, :, sl, :]               # [16, 512 l, 512 d]
        # [p, t*2048 + c*512 + l] = ms[t, l, c*128 + p]
        mf = np.ascontiguousarray(
            ms.transpose(2, 0, 1)                    # [512 f, 16, 512 l]
            .reshape(4, 128, T, LH)                  # [c, p, t, l]
            .transpose(1, 2, 0, 3)                   # [p, t, c, l]
            .reshape(128, T * 2048))
        m["MEMF8"] = mf.astype(ml_dtypes.float8_e4m3)
        m["M15D"] = _bf(np.ascontiguousarray(
            ms[15].T.reshape(4, 128, LH).transpose(1, 0, 2).reshape(128, 2048)))
        in_maps.append(m)
    return in_maps


def kernel(**inputs):
    if "nc" not in _CACHE:
        _CACHE["nc"] = _build()
    ncb = _CACHE["nc"]
    in_maps = _prep_maps(inputs)
    res = bass_utils.run_bass_kernel_spmd(ncb, in_maps, core_ids=list(range(NC)))
    out = np.empty((B, L, D), np.float32)
    for k in range(NC):
        bb, h = k // 2, k % 2
        o = np.asarray(res.results[k]["OUT"], np.float32)   # [4,128,512] fm
        out[bb, h * LH:(h + 1) * LH, :] = o.transpose(2, 0, 1).reshape(LH, D)
    return out
